# revision 6
# baseline (speedup 1.0000x reference)
"""Trainium2 Bass kernel for nn_DecoderRNN: greedy LSTM decoder, 8-way model-parallel.

Strategy (48 sequential decode steps, batch=16 kept whole on every core):
  - Big weights SBUF-resident, sharded 8 ways: W_hh^T gate-slice (2MB),
    W'^T vocab-slice (2MB), W1^T vocab-slice (8.4MB), L^T (4MB, Cholesky factor
    of Gram(W') used to compute var(y) locally without communication).
  - x @ W_ih^T precomputed on host for all 4096 vocab rows (embgates table in
    HBM); per step it is an indirect-DMA row gather keyed by the argmax.
  - LN(y): mean folded into W' columns; 1/sigma from ||h L^T||^2 = h^T G h.
  - 3 small AllGathers per step: h^T (8KB/core), z^T (32KB/core), stats (64B).
  - softmax: device emits unnormalized exp + per-step partial sums; host divides.
Matmuls are fp32, activation-stationary (lhsT = activations [128,16]), 4-way
column-tiled across PE col-groups, partials recombined by a selector matmul.
"""
import numpy as np
import sys

sys.path.insert(0, "/opt/trn_rl_repo")

import concourse.bacc as bacc
import concourse.mybir as mybir
import concourse.tile as tile
from concourse.bass import IndirectOffsetOnAxis
from concourse.bass_utils import run_bass_kernel_spmd

f32 = mybir.dt.float32
i32 = mybir.dt.int32
AF = mybir.ActivationFunctionType
ALU = mybir.AluOpType
AX = mybir.AxisListType

V, E, H, B, T = 4096, 512, 1024, 16, 48
NCORE = 8
VS = V // NCORE      # 512 vocab rows per core
HS = H // NCORE      # 128 h dims per core
GS = 4 * H // NCORE  # 512 gate rows per core
KH = H // 128        # 8 contraction k-tiles over H
KV = V // 128        # 32 contraction k-tiles over V
ES = E // NCORE      # 64 embed cols per core
EPS = 1e-5

_cache = {}


def _build(t_steps):
    nc = bacc.Bacc("TRN2", target_bir_lowering=False, debug=False, num_devices=NCORE)

    # ---- per-core inputs ----
    whhT_in = nc.dram_tensor("whhT_in", [128, KH * GS], f32, kind="ExternalInput")
    wpT_in = nc.dram_tensor("wpT_in", [128, KH * VS], f32, kind="ExternalInput")
    lT_in = nc.dram_tensor("lT_in", [128, KH * H], f32, kind="ExternalInput")
    w1T_in = nc.dram_tensor("w1T_in", [128, KV * VS], f32, kind="ExternalInput")
    embg_in = nc.dram_tensor("embg_in", [V, GS], f32, kind="ExternalInput")
    embe_in = nc.dram_tensor("embe_in", [V, ES], f32, kind="ExternalInput")
    gates0_in = nc.dram_tensor("gates0_in", [16, GS], f32, kind="ExternalInput")
    sel_in = nc.dram_tensor("sel_in", [128, 16], f32, kind="ExternalInput")
    id16_in = nc.dram_tensor("id16_in", [16, 16], f32, kind="ExternalInput")
    iotag_in = nc.dram_tensor("iotag_in", [16, VS], f32, kind="ExternalInput")
    big_in = nc.dram_tensor("big_in", [16, VS], f32, kind="ExternalInput")

    # ---- per-core outputs ----
    q_out = nc.dram_tensor("q_out", [t_steps * 16, VS], f32, kind="ExternalOutput")
    se_out = nc.dram_tensor("se_out", [16, t_steps], f32, kind="ExternalOutput")
    hs_out = nc.dram_tensor("hs_out", [16, t_steps * HS], f32, kind="ExternalOutput")
    ge_out = nc.dram_tensor("ge_out", [16, t_steps * ES], f32, kind="ExternalOutput")
    pr_out = nc.dram_tensor("pr_out", [16, t_steps], f32, kind="ExternalOutput")

    with tile.TileContext(nc) as tc:
        with (
            tc.tile_pool(name="wres", bufs=1) as wres,          # resident weights/consts
            tc.tile_pool(name="state", bufs=1) as state,        # persistent state
            tc.tile_pool(name="work", bufs=2) as work,          # per-step pipeline tiles
            tc.tile_pool(name="scr", bufs=1) as scrp,           # scratch
            tc.tile_pool(name="ps_big", bufs=2, space="PSUM") as ps_big,
            tc.tile_pool(name="ps_out", bufs=3, space="PSUM") as ps_out,
            tc.tile_pool(name="ps_tr", bufs=2, space="PSUM") as ps_tr,
            tc.tile_pool(name="dram", bufs=2, space="DRAM") as dram,
        ):
            # resident loads
            whhT = wres.tile([128, KH * GS], f32)
            wpT = wres.tile([128, KH * VS], f32)
            lT = wres.tile([128, KH * H], f32)
            w1T = wres.tile([128, KV * VS], f32)
            sel = wres.tile([128, 16], f32)
            id16 = wres.tile([16, 16], f32)
            iotag = wres.tile([16, VS], f32)
            bigv = wres.tile([16, VS], f32)
            for dst, src in (
                (whhT, whhT_in), (wpT, wpT_in), (lT, lT_in), (w1T, w1T_in),
                (sel, sel_in), (id16, id16_in), (iotag, iotag_in), (bigv, big_in),
            ):
                nc.sync.dma_start(dst[:], src[:])

            # persistent state
            c_st = state.tile([16, HS], f32)
            nc.vector.memset(c_st[:], 0.0)
            epsv = state.tile([16, 1], f32)
            nc.vector.memset(epsv[:], EPS)
            ge_all = state.tile([16, t_steps * ES], f32)
            se_all = state.tile([16, t_steps], f32)
            pr_all = state.tile([16, t_steps], f32)
            htiles = state.tile([128, KH * 16], f32)    # gathered h^T
            ztiles = state.tile([128, KV * 16], f32)    # gathered z^T
            xg = state.tile([16, GS], f32)              # x-gates (embgates row / gates0)
            nc.sync.dma_start(xg[:], gates0_in[:])

            def mm4(out_ps, lhs_tiles, rhs, ktiles, n_out, rhs_col0=0, rhs_stride=None):
                """4-way col-tiled matmul, contraction split over col groups."""
                if rhs_stride is None:
                    rhs_stride = n_out
                per = ktiles // 4
                for kk in range(per):
                    for j in range(4):
                        kt = j * per + kk
                        c0 = rhs_col0 + kt * rhs_stride
                        nc.tensor.matmul(
                            out_ps[32 * j : 32 * j + 16, 0:n_out],
                            lhs_tiles(kt),
                            rhs[:, c0 : c0 + n_out],
                            start=(kk == 0),
                            stop=(kk == per - 1),
                            tile_position=(0, 32 * j),
                        )

            def selsum(big_ps, n_out):
                """sum the 4 col-group partials -> [16,n_out] PSUM via selector matmul"""
                sb_t = work.tile([128, 512], f32, tag="selsb")
                nc.vector.tensor_copy(sb_t[:, 0:n_out], big_ps[:, 0:n_out])
                out_ps_t = ps_out.tile([16, 512], f32, tag="selout")
                nc.tensor.matmul(out_ps_t[:, 0:n_out], sel[:], sb_t[:, 0:n_out],
                                 start=True, stop=True)
                return out_ps_t

            for t in range(t_steps):
                # ---------- gates ----------
                if t == 0:
                    gates = xg
                else:
                    ghh_ps = ps_big.tile([128, 512], f32, tag="big")
                    mm4(ghh_ps, lambda kt: htiles[:, kt * 16 : (kt + 1) * 16], whhT, KH, GS)
                    gsum = selsum(ghh_ps, GS)
                    gates = work.tile([16, GS], f32, tag="gates")
                    nc.vector.tensor_add(gates[:], gsum[:, 0:GS], xg[:])

                # ---------- LSTM elementwise (gate order [i|f|o|g]) ----------
                sfo = scrp.tile([16, 384], f32, tag="sfo")
                nc.scalar.activation(sfo[:], gates[:, 0:384], AF.Sigmoid)
                tg = scrp.tile([16, HS], f32, tag="tg")
                nc.scalar.activation(tg[:], gates[:, 384:512], AF.Tanh)
                m1 = scrp.tile([16, HS], f32, tag="m1")
                nc.vector.tensor_mul(m1[:], sfo[:, 128:256], c_st[:])
                m2 = scrp.tile([16, HS], f32, tag="m2")
                nc.vector.tensor_mul(m2[:], sfo[:, 0:128], tg[:])
                nc.vector.tensor_add(c_st[:], m1[:], m2[:])
                tc_t = scrp.tile([16, HS], f32, tag="tc")
                nc.scalar.activation(tc_t[:], c_st[:], AF.Tanh)
                h_sb = work.tile([16, HS], f32, tag="h")
                nc.vector.tensor_mul(h_sb[:], sfo[:, 256:384], tc_t[:])
                nc.sync.dma_start(hs_out[:, t * HS : (t + 1) * HS], h_sb[:])

                # ---------- h^T exchange ----------
                htp = ps_tr.tile([128, 16], f32, tag="tr")
                nc.tensor.transpose(htp[:], h_sb[:], id16[:])
                hstage = work.tile([128, 16], f32, tag="hstage")
                nc.vector.tensor_copy(hstage[:], htp[:])
                hag_i = dram.tile([128, 16], f32, tag="hag_i")
                hag_o = dram.tile([128 * NCORE, 16], f32, tag="hag_o")
                nc.sync.dma_start(hag_i[:], hstage[:])
                nc.gpsimd.collective_compute(
                    "AllGather", ALU.bypass,
                    ins=[hag_i[:].opt()], outs=[hag_o[:].opt()],
                    replica_groups=[list(range(NCORE))],
                )
                nc.sync.dma_start(
                    htiles[:], hag_o[:].rearrange("(r p) b -> p r b", p=128)
                )

                # ---------- y = h @ W'^T  (gives y - mean(y)) ----------
                y_ps = ps_big.tile([128, 512], f32, tag="big")
                mm4(y_ps, lambda kt: htiles[:, kt * 16 : (kt + 1) * 16], wpT, KH, VS)
                ym = selsum(y_ps, VS)

                # ---------- ss = ||h L^T||^2 -> inv sigma ----------
                ssh = scrp.tile([16, 2], f32, tag="ssh")
                for hh in range(2):
                    l_ps = ps_big.tile([128, 512], f32, tag="big")
                    mm4(l_ps, lambda kt: htiles[:, kt * 16 : (kt + 1) * 16], lT, KH, 512,
                        rhs_col0=hh * 512, rhs_stride=H)
                    wsum = selsum(l_ps, 512)
                    wscr = scrp.tile([16, 512], f32, tag="scr512")
                    nc.scalar.activation(wscr[:], wsum[:, 0:512], AF.Square,
                                         accum_out=ssh[:, hh : hh + 1])
                ss = scrp.tile([16, 1], f32, tag="ss")
                nc.vector.reduce_sum(ss[:], ssh[:], axis=AX.X)
                rr = scrp.tile([16, 1], f32, tag="rr")
                nc.scalar.activation(rr[:], ss[:], AF.Sqrt, bias=epsv[:], scale=1.0 / V)
                invs = scrp.tile([16, 1], f32, tag="invs")
                nc.vector.reciprocal(invs[:], rr[:])

                # ---------- z = lrelu_0.3(y * invs) ----------
                zn = scrp.tile([16, VS], f32, tag="zn")
                nc.vector.tensor_scalar(zn[:], ym[:, 0:VS], invs[:], None, op0=ALU.mult)
                z_sb = work.tile([16, VS], f32, tag="z")
                nc.vector.scalar_tensor_tensor(z_sb[:], zn[:], 0.3, zn[:],
                                               op0=ALU.mult, op1=ALU.max)

                # ---------- z^T exchange ----------
                zstage = work.tile([128, 4 * 16], f32, tag="zstage")
                for tt in range(4):
                    ztp = ps_tr.tile([128, 16], f32, tag="tr")
                    nc.tensor.transpose(ztp[:], z_sb[:, tt * 128 : (tt + 1) * 128], id16[:])
                    nc.vector.tensor_copy(zstage[:, tt * 16 : (tt + 1) * 16], ztp[:])
                zag_i = dram.tile([128, 64], f32, tag="zag_i")
                zag_o = dram.tile([128 * NCORE, 64], f32, tag="zag_o")
                nc.sync.dma_start(zag_i[:], zstage[:])
                nc.gpsimd.collective_compute(
                    "AllGather", ALU.bypass,
                    ins=[zag_i[:].opt()], outs=[zag_o[:].opt()],
                    replica_groups=[list(range(NCORE))],
                )
                nc.sync.dma_start(
                    ztiles[:], zag_o[:].rearrange("(r p) (tt b) -> p r tt b", p=128, b=16)
                )

                # ---------- y1 = z @ W1^T ----------
                y1_ps = ps_big.tile([128, 512], f32, tag="big")
                mm4(y1_ps, lambda kt: ztiles[:, kt * 16 : (kt + 1) * 16], w1T, KV, VS)
                y1m = selsum(y1_ps, VS)

                # ---------- local stats (sum, sumsq, max, first-argmax) ----------
                stats = work.tile([16, 4], f32, tag="stats")
                nc.vector.reduce_sum(stats[:, 0:1], y1m[:, 0:VS], axis=AX.X)
                sscr = scrp.tile([16, 512], f32, tag="scr512")
                nc.scalar.activation(sscr[:], y1m[:, 0:VS], AF.Square,
                                     accum_out=stats[:, 1:2])
                nc.vector.reduce_max(stats[:, 2:3], y1m[:, 0:VS], axis=AX.X)
                msk = scrp.tile([16, VS], mybir.dt.uint8, tag="msk")
                nc.vector.tensor_scalar(msk[:], y1m[:, 0:VS], stats[:, 2:3], None,
                                        op0=ALU.is_equal)
                mi = scrp.tile([16, VS], f32, tag="mi")
                nc.vector.select(mi[:], msk[:], iotag[:], bigv[:])
                nc.vector.tensor_reduce(stats[:, 3:4], mi[:], op=ALU.min, axis=AX.X)

                # ---------- stats exchange ----------
                sag_i = dram.tile([16, 4], f32, tag="sag_i")
                sag_o = dram.tile([16 * NCORE, 4], f32, tag="sag_o")
                nc.sync.dma_start(sag_i[:], stats[:])
                nc.gpsimd.collective_compute(
                    "AllGather", ALU.bypass,
                    ins=[sag_i[:].opt()], outs=[sag_o[:].opt()],
                    replica_groups=[list(range(NCORE))],
                )
                stg = work.tile([16, NCORE * 4], f32, tag="stg")
                nc.sync.dma_start(
                    stg[:], sag_o[:].rearrange("(r b) j -> b r j", b=16)
                )
                stv = stg[:].rearrange("b (r j) -> b j r", j=4)  # [16, 4, 8]

                # ---------- combine ----------
                s_tot = scrp.tile([16, 1], f32, tag="s_tot")
                nc.vector.reduce_sum(s_tot[:], stv[:, 0, :], axis=AX.X)
                q_tot = scrp.tile([16, 1], f32, tag="q_tot")
                nc.vector.reduce_sum(q_tot[:], stv[:, 1, :], axis=AX.X)
                m_tot = scrp.tile([16, 1], f32, tag="m_tot")
                nc.vector.reduce_max(m_tot[:], stv[:, 2, :], axis=AX.X)
                msk2 = scrp.tile([16, NCORE], mybir.dt.uint8, tag="msk2")
                nc.vector.tensor_scalar(msk2[:], stv[:, 2, :], m_tot[:], None,
                                        op0=ALU.is_equal)
                mi2 = scrp.tile([16, NCORE], f32, tag="mi2")
                nc.vector.select(mi2[:], msk2[:], stv[:, 3, :], bigv[:, 0:NCORE])
                pred = pr_all[:, t : t + 1]
                nc.vector.tensor_reduce(pred, mi2[:], op=ALU.min, axis=AX.X)
                idx_t = work.tile([16, 1], i32, tag="idx")
                nc.vector.tensor_copy(idx_t[:], pred)

                # ---------- LN1 scalars + unnormalized softmax ----------
                mu1 = scrp.tile([16, 1], f32, tag="mu1")
                nc.vector.tensor_scalar(mu1[:], s_tot[:], 1.0 / V, None, op0=ALU.mult)
                nmu2 = scrp.tile([16, 1], f32, tag="nmu2")
                nc.vector.scalar_tensor_tensor(nmu2[:], mu1[:], -1.0, mu1[:],
                                               op0=ALU.mult, op1=ALU.mult)
                var1 = scrp.tile([16, 1], f32, tag="var1")
                nc.vector.scalar_tensor_tensor(var1[:], q_tot[:], 1.0 / V, nmu2[:],
                                               op0=ALU.mult, op1=ALU.add)
                r1 = scrp.tile([16, 1], f32, tag="r1")
                nc.scalar.activation(r1[:], var1[:], AF.Sqrt, bias=epsv[:], scale=1.0)
                invs1 = scrp.tile([16, 1], f32, tag="invs1")
                nc.vector.reciprocal(invs1[:], r1[:])
                b1 = scrp.tile([16, 1], f32, tag="b1")
                nc.vector.scalar_tensor_tensor(b1[:], mu1[:], -1.0, invs1[:],
                                               op0=ALU.mult, op1=ALU.mult)
                ex = work.tile([16, VS], f32, tag="ex")
                nc.scalar.activation(ex[:], y1m[:, 0:VS], AF.Exp, bias=b1[:],
                                     scale=invs1[:], accum_out=se_all[:, t : t + 1])
                nc.sync.dma_start(q_out[t * 16 : (t + 1) * 16, :], ex[:])

                # ---------- next-step gathers ----------
                nc.gpsimd.indirect_dma_start(
                    ge_all[:, t * ES : (t + 1) * ES], None, embe_in[:],
                    IndirectOffsetOnAxis(ap=idx_t[:, 0:1], axis=0),
                )
                if t + 1 < t_steps:
                    nc.gpsimd.indirect_dma_start(
                        xg[:], None, embg_in[:],
                        IndirectOffsetOnAxis(ap=idx_t[:, 0:1], axis=0),
                    )

            # ---------- final output DMAs ----------
            nc.sync.dma_start(ge_out[:], ge_all[:])
            nc.sync.dma_start(se_out[:], se_all[:])
            nc.sync.dma_start(pr_out[:], pr_all[:])

    nc.compile()
    return nc


def _host_prep(inputs):
    fp = np.float32
    feats = np.asarray(inputs["features"], fp)
    emb_t = np.asarray(inputs["embed_table"], fp)
    W_ih = np.asarray(inputs["W_ih"], fp)
    W_hh = np.asarray(inputs["W_hh"], fp)
    b_ih = np.asarray(inputs["b_ih"], fp)
    b_hh = np.asarray(inputs["b_hh"], fp)
    lin_v = np.asarray(inputs["lin_v"], fp)
    lin_g = np.asarray(inputs["lin_g"], fp)
    lin1_v = np.asarray(inputs["lin1_v"], fp)
    lin1_g = np.asarray(inputs["lin1_g"], fp)

    W = (lin_g[:, None] * lin_v / np.linalg.norm(lin_v, axis=1, keepdims=True)).astype(fp)
    W1 = (lin1_g[:, None] * lin1_v / np.linalg.norm(lin1_v, axis=1, keepdims=True)).astype(fp)
    Wp = (W - W.mean(axis=0, keepdims=True)).astype(fp)
    G = Wp.astype(np.float64).T @ Wp.astype(np.float64)
    try:
        L = np.linalg.cholesky(G)
    except np.linalg.LinAlgError:
        L = np.linalg.cholesky(G + 1e-9 * np.eye(H))
    L = L.astype(fp)

    embgates = (emb_t @ W_ih.T + b_ih + b_hh).astype(fp)   # (V, 4H)
    gates0 = (feats @ W_ih.T + b_ih + b_hh).astype(fp)     # (B, 4H)

    sel = np.zeros((128, 16), fp)
    for j in range(4):
        for bb in range(16):
            sel[32 * j + bb, bb] = 1.0
    id16 = np.eye(16, dtype=fp)
    big = np.full((16, VS), 1e30, fp)

    in_maps = []
    for k in range(NCORE):
        rows = np.concatenate([
            np.arange(k * HS, (k + 1) * HS),                # i
            H + np.arange(k * HS, (k + 1) * HS),            # f
            3 * H + np.arange(k * HS, (k + 1) * HS),        # o
            2 * H + np.arange(k * HS, (k + 1) * HS),        # g
        ])
        whh_k = W_hh[rows]                                  # (GS, H)
        whhT = np.ascontiguousarray(
            whh_k.T.reshape(KH, 128, GS).transpose(1, 0, 2).reshape(128, KH * GS)
        )
        wp_k = Wp[k * VS : (k + 1) * VS]                    # (VS, H)
        wpT = np.ascontiguousarray(
            wp_k.T.reshape(KH, 128, VS).transpose(1, 0, 2).reshape(128, KH * VS)
        )
        lT = np.ascontiguousarray(
            L.T.reshape(KH, 128, H).transpose(1, 0, 2).reshape(128, KH * H)
        )
        w1_k = W1[k * VS : (k + 1) * VS]                    # (VS, V)
        w1T = np.ascontiguousarray(
            w1_k.T.reshape(KV, 128, VS).transpose(1, 0, 2).reshape(128, KV * VS)
        )
        iotag = np.broadcast_to(
            (k * VS + np.arange(VS)).astype(fp)[None, :], (16, VS)
        ).copy()
        in_maps.append({
            "whhT_in": whhT, "wpT_in": wpT, "lT_in": lT, "w1T_in": w1T,
            "embg_in": np.ascontiguousarray(embgates[:, rows]),
            "embe_in": np.ascontiguousarray(emb_t[:, k * ES : (k + 1) * ES]),
            "gates0_in": np.ascontiguousarray(gates0[:, rows]),
            "sel_in": sel, "id16_in": id16, "iotag_in": iotag, "big_in": big,
        })
    return in_maps


def kernel(**inputs):
    t_steps = int(np.asarray(inputs["captions"]).shape[1])
    in_maps = _host_prep(inputs)
    if t_steps not in _cache:
        _cache[t_steps] = _build(t_steps)
    nc = _cache[t_steps]
    import os as _os
    _tr = _os.environ.get("BASS_KERNEL_TRACE") == "1"
    res = run_bass_kernel_spmd(nc, in_maps, core_ids=list(range(NCORE)), trace=_tr)
    global LAST_RESULT
    LAST_RESULT = res
    rs = res.results

    fp = np.float32
    q_slices = np.stack([r["q_out"].reshape(t_steps, 16, VS) for r in rs], axis=0)
    q_full = q_slices.transpose(2, 1, 0, 3).reshape(16, t_steps, V)
    sumexp = np.sum(np.stack([r["se_out"] for r in rs]), axis=0)  # (16, T)
    Q = (q_full / sumexp[:, :, None]).astype(fp)
    hs = np.concatenate(
        [r["hs_out"].reshape(16, t_steps, HS) for r in rs], axis=2
    ).astype(fp)
    ge = np.concatenate(
        [r["ge_out"].reshape(16, t_steps, ES) for r in rs], axis=2
    ).astype(fp)
    preds = rs[0]["pr_out"].astype(fp)
    return (
        Q.reshape(B, t_steps, 1, 1, V),
        hs.reshape(B, t_steps, 1, 1, H),
        ge.reshape(B, t_steps, 1, 1, E),
        preds.reshape(B, t_steps, 1),
    )


# revision 7
# speedup vs baseline: 1.0194x; 1.0194x over previous
"""Trainium2 Bass kernel for nn_DecoderRNN: greedy LSTM decoder, 8-way model-parallel.

Strategy (48 sequential decode steps, batch=16 kept whole on every core):
  - Big weights SBUF-resident, sharded 8 ways: W_hh^T gate-slice (2MB),
    W'^T vocab-slice (2MB), W1^T vocab-slice (8.4MB), L^T (4MB, Cholesky factor
    of Gram(W') used to compute var(y) locally without communication).
  - x @ W_ih^T precomputed on host for all 4096 vocab rows (embgates table in
    HBM); per step it is an indirect-DMA row gather keyed by the argmax.
  - LN(y): mean folded into W' columns; 1/sigma from ||h L^T||^2 = h^T G h.
  - 3 small AllGathers per step: h^T (8KB/core), z^T (32KB/core), stats (64B).
  - softmax: device emits unnormalized exp + per-step partial sums; host divides.
Matmuls are fp32, activation-stationary (lhsT = activations [128,16]), 4-way
column-tiled across PE col-groups, partials recombined by a selector matmul.
"""
import numpy as np
import sys

sys.path.insert(0, "/opt/trn_rl_repo")

import concourse.bacc as bacc
import concourse.mybir as mybir
import concourse.tile as tile
from concourse.bass import IndirectOffsetOnAxis
from concourse.bass_utils import run_bass_kernel_spmd

f32 = mybir.dt.float32
i32 = mybir.dt.int32
AF = mybir.ActivationFunctionType
ALU = mybir.AluOpType
AX = mybir.AxisListType

V, E, H, B, T = 4096, 512, 1024, 16, 48
NCORE = 8
VS = V // NCORE      # 512 vocab rows per core
HS = H // NCORE      # 128 h dims per core
GS = 4 * H // NCORE  # 512 gate rows per core
KH = H // 128        # 8 contraction k-tiles over H
KV = V // 128        # 32 contraction k-tiles over V
ES = E // NCORE      # 64 embed cols per core
EPS = 1e-5

_cache = {}
NOCOLL = __import__("os").environ.get("KERNEL_NOCOLL") == "1"


def _ag(nc, in_t, out_t, nrep):
    if NOCOLL:
        # local stand-in: copy own block into slot 0 (timing bisection only)
        nc.sync.dma_start(out_t[0 : in_t.shape[0], :], in_t[:])
    else:
        nc.gpsimd.collective_compute(
            "AllGather", mybir.AluOpType.bypass,
            ins=[in_t[:].opt()], outs=[out_t[:].opt()],
            replica_groups=[list(range(nrep))],
        )


def _build(t_steps):
    nc = bacc.Bacc("TRN2", target_bir_lowering=False, debug=False, num_devices=NCORE)

    # ---- per-core inputs ----
    whhT_in = nc.dram_tensor("whhT_in", [128, KH * GS], f32, kind="ExternalInput")
    wpT_in = nc.dram_tensor("wpT_in", [128, KH * VS], f32, kind="ExternalInput")
    lT_in = nc.dram_tensor("lT_in", [128, KH * H], f32, kind="ExternalInput")
    w1T_in = nc.dram_tensor("w1T_in", [128, KV * VS], f32, kind="ExternalInput")
    embg_in = nc.dram_tensor("embg_in", [V, GS], f32, kind="ExternalInput")
    embe_in = nc.dram_tensor("embe_in", [V, ES], f32, kind="ExternalInput")
    gates0_in = nc.dram_tensor("gates0_in", [16, GS], f32, kind="ExternalInput")
    sel_in = nc.dram_tensor("sel_in", [128, 16], f32, kind="ExternalInput")
    id16_in = nc.dram_tensor("id16_in", [16, 16], f32, kind="ExternalInput")
    iotag_in = nc.dram_tensor("iotag_in", [16, VS], f32, kind="ExternalInput")
    big_in = nc.dram_tensor("big_in", [16, VS], f32, kind="ExternalInput")

    # ---- per-core outputs ----
    q_out = nc.dram_tensor("q_out", [t_steps * 16, VS], f32, kind="ExternalOutput")
    se_out = nc.dram_tensor("se_out", [16, t_steps], f32, kind="ExternalOutput")
    hs_out = nc.dram_tensor("hs_out", [16, t_steps * HS], f32, kind="ExternalOutput")
    ge_out = nc.dram_tensor("ge_out", [16, t_steps * ES], f32, kind="ExternalOutput")
    pr_out = nc.dram_tensor("pr_out", [16, t_steps], f32, kind="ExternalOutput")

    with tile.TileContext(nc) as tc:
        with (
            tc.tile_pool(name="wres", bufs=1) as wres,          # resident weights/consts
            tc.tile_pool(name="state", bufs=1) as state,        # persistent state
            tc.tile_pool(name="work", bufs=2) as work,          # per-step pipeline tiles
            tc.tile_pool(name="scr", bufs=1) as scrp,           # scratch
            tc.tile_pool(name="ps_big", bufs=2, space="PSUM") as ps_big,
            tc.tile_pool(name="ps_out", bufs=3, space="PSUM") as ps_out,
            tc.tile_pool(name="ps_tr", bufs=2, space="PSUM") as ps_tr,
            tc.tile_pool(name="dram", bufs=2, space="DRAM") as dram,
        ):
            # resident loads
            whhT = wres.tile([128, KH * GS], f32)
            wpT = wres.tile([128, KH * VS], f32)
            lT = wres.tile([128, KH * H], f32)
            w1T = wres.tile([128, KV * VS], f32)
            sel = wres.tile([128, 16], f32)
            id16 = wres.tile([16, 16], f32)
            iotag = wres.tile([16, VS], f32)
            bigv = wres.tile([16, VS], f32)
            for dst, src in (
                (whhT, whhT_in), (wpT, wpT_in), (lT, lT_in), (w1T, w1T_in),
                (sel, sel_in), (id16, id16_in), (iotag, iotag_in), (bigv, big_in),
            ):
                nc.sync.dma_start(dst[:], src[:])

            # persistent state
            c_st = state.tile([16, HS], f32)
            nc.vector.memset(c_st[:], 0.0)
            epsv = state.tile([16, 1], f32)
            nc.vector.memset(epsv[:], EPS)
            ge_all = state.tile([16, t_steps * ES], f32)
            se_all = state.tile([16, t_steps], f32)
            pr_all = state.tile([16, t_steps], f32)
            htiles = state.tile([128, KH * 16], f32)    # gathered h^T
            ztiles = state.tile([128, KV * 16], f32)    # gathered z^T
            xg = state.tile([16, GS], f32)              # x-gates (embgates row / gates0)
            nc.sync.dma_start(xg[:], gates0_in[:])

            def mm4(out_ps, lhs_tiles, rhs, ktiles, n_out, rhs_col0=0, rhs_stride=None):
                """4-way col-tiled matmul, contraction split over col groups."""
                if rhs_stride is None:
                    rhs_stride = n_out
                per = ktiles // 4
                for kk in range(per):
                    for j in range(4):
                        kt = j * per + kk
                        c0 = rhs_col0 + kt * rhs_stride
                        nc.tensor.matmul(
                            out_ps[32 * j : 32 * j + 16, 0:n_out],
                            lhs_tiles(kt),
                            rhs[:, c0 : c0 + n_out],
                            start=(kk == 0),
                            stop=(kk == per - 1),
                            tile_position=(0, 32 * j),
                        )

            def selsum(big_ps, n_out):
                """sum the 4 col-group partials -> [16,n_out] PSUM via selector matmul"""
                sb_t = work.tile([128, 512], f32, tag="selsb")
                nc.vector.tensor_copy(sb_t[:, 0:n_out], big_ps[:, 0:n_out])
                out_ps_t = ps_out.tile([16, 512], f32, tag="selout")
                nc.tensor.matmul(out_ps_t[:, 0:n_out], sel[:], sb_t[:, 0:n_out],
                                 start=True, stop=True)
                return out_ps_t

            for t in range(t_steps):
                # ---------- gates ----------
                if t == 0:
                    gates = xg
                else:
                    ghh_ps = ps_big.tile([128, 512], f32, tag="big")
                    mm4(ghh_ps, lambda kt: htiles[:, kt * 16 : (kt + 1) * 16], whhT, KH, GS)
                    gsum = selsum(ghh_ps, GS)
                    gates = work.tile([16, GS], f32, tag="gates")
                    nc.vector.tensor_add(gates[:], gsum[:, 0:GS], xg[:])

                # ---------- LSTM elementwise (gate order [i|f|o|g]) ----------
                sfo = scrp.tile([16, 384], f32, tag="sfo")
                nc.scalar.activation(sfo[:], gates[:, 0:384], AF.Sigmoid)
                tg = scrp.tile([16, HS], f32, tag="tg")
                nc.scalar.activation(tg[:], gates[:, 384:512], AF.Tanh)
                m1 = scrp.tile([16, HS], f32, tag="m1")
                nc.vector.tensor_mul(m1[:], sfo[:, 128:256], c_st[:])
                m2 = scrp.tile([16, HS], f32, tag="m2")
                nc.vector.tensor_mul(m2[:], sfo[:, 0:128], tg[:])
                nc.vector.tensor_add(c_st[:], m1[:], m2[:])
                tc_t = scrp.tile([16, HS], f32, tag="tc")
                nc.scalar.activation(tc_t[:], c_st[:], AF.Tanh)
                h_sb = work.tile([16, HS], f32, tag="h")
                nc.vector.tensor_mul(h_sb[:], sfo[:, 256:384], tc_t[:])
                nc.sync.dma_start(hs_out[:, t * HS : (t + 1) * HS], h_sb[:])

                # ---------- h^T exchange ----------
                htp = ps_tr.tile([128, 16], f32, tag="tr")
                nc.tensor.transpose(htp[:], h_sb[:], id16[:])
                hstage = work.tile([128, 16], f32, tag="hstage")
                nc.vector.tensor_copy(hstage[:], htp[:])
                hag_i = dram.tile([128, 16], f32, tag="hag_i")
                hag_o = dram.tile([128 * NCORE, 16], f32, tag="hag_o")
                nc.sync.dma_start(hag_i[:], hstage[:])
                _ag(nc, hag_i, hag_o, NCORE)
                nc.sync.dma_start(
                    htiles[:], hag_o[:].rearrange("(r p) b -> p r b", p=128)
                )

                # ---------- y = h @ W'^T  (gives y - mean(y)) ----------
                y_ps = ps_big.tile([128, 512], f32, tag="big")
                mm4(y_ps, lambda kt: htiles[:, kt * 16 : (kt + 1) * 16], wpT, KH, VS)
                ym = selsum(y_ps, VS)

                # ---------- ss = ||h L^T||^2 -> inv sigma ----------
                ssh = scrp.tile([16, 2], f32, tag="ssh")
                for hh in range(2):
                    l_ps = ps_big.tile([128, 512], f32, tag="big")
                    mm4(l_ps, lambda kt: htiles[:, kt * 16 : (kt + 1) * 16], lT, KH, 512,
                        rhs_col0=hh * 512, rhs_stride=H)
                    wsum = selsum(l_ps, 512)
                    wscr = scrp.tile([16, 512], f32, tag="scr512")
                    nc.scalar.activation(wscr[:], wsum[:, 0:512], AF.Square,
                                         accum_out=ssh[:, hh : hh + 1])
                ss = scrp.tile([16, 1], f32, tag="ss")
                nc.vector.reduce_sum(ss[:], ssh[:], axis=AX.X)
                rr = scrp.tile([16, 1], f32, tag="rr")
                nc.scalar.activation(rr[:], ss[:], AF.Sqrt, bias=epsv[:], scale=1.0 / V)
                invs = scrp.tile([16, 1], f32, tag="invs")
                nc.vector.reciprocal(invs[:], rr[:])

                # ---------- z = lrelu_0.3(y * invs) ----------
                zn = scrp.tile([16, VS], f32, tag="zn")
                nc.vector.tensor_scalar(zn[:], ym[:, 0:VS], invs[:], None, op0=ALU.mult)
                z_sb = work.tile([16, VS], f32, tag="z")
                nc.vector.scalar_tensor_tensor(z_sb[:], zn[:], 0.3, zn[:],
                                               op0=ALU.mult, op1=ALU.max)

                # ---------- z^T exchange ----------
                zstage = work.tile([128, 4 * 16], f32, tag="zstage")
                for tt in range(4):
                    ztp = ps_tr.tile([128, 16], f32, tag="tr")
                    nc.tensor.transpose(ztp[:], z_sb[:, tt * 128 : (tt + 1) * 128], id16[:])
                    nc.vector.tensor_copy(zstage[:, tt * 16 : (tt + 1) * 16], ztp[:])
                zag_i = dram.tile([128, 64], f32, tag="zag_i")
                zag_o = dram.tile([128 * NCORE, 64], f32, tag="zag_o")
                nc.sync.dma_start(zag_i[:], zstage[:])
                _ag(nc, zag_i, zag_o, NCORE)
                nc.sync.dma_start(
                    ztiles[:], zag_o[:].rearrange("(r p) (tt b) -> p r tt b", p=128, b=16)
                )

                # ---------- y1 = z @ W1^T ----------
                y1_ps = ps_big.tile([128, 512], f32, tag="big")
                mm4(y1_ps, lambda kt: ztiles[:, kt * 16 : (kt + 1) * 16], w1T, KV, VS)
                y1m = selsum(y1_ps, VS)

                # ---------- local stats (sum, sumsq, max, first-argmax) ----------
                stats = work.tile([16, 4], f32, tag="stats")
                nc.vector.reduce_sum(stats[:, 0:1], y1m[:, 0:VS], axis=AX.X)
                sscr = scrp.tile([16, 512], f32, tag="scr512")
                nc.scalar.activation(sscr[:], y1m[:, 0:VS], AF.Square,
                                     accum_out=stats[:, 1:2])
                nc.vector.reduce_max(stats[:, 2:3], y1m[:, 0:VS], axis=AX.X)
                msk = scrp.tile([16, VS], mybir.dt.uint8, tag="msk")
                nc.vector.tensor_scalar(msk[:], y1m[:, 0:VS], stats[:, 2:3], None,
                                        op0=ALU.is_equal)
                mi = scrp.tile([16, VS], f32, tag="mi")
                nc.vector.select(mi[:], msk[:], iotag[:], bigv[:])
                nc.vector.tensor_reduce(stats[:, 3:4], mi[:], op=ALU.min, axis=AX.X)

                # ---------- stats exchange ----------
                sag_i = dram.tile([16, 4], f32, tag="sag_i")
                sag_o = dram.tile([16 * NCORE, 4], f32, tag="sag_o")
                nc.sync.dma_start(sag_i[:], stats[:])
                _ag(nc, sag_i, sag_o, NCORE)
                stg = work.tile([16, NCORE * 4], f32, tag="stg")
                nc.sync.dma_start(
                    stg[:], sag_o[:].rearrange("(r b) j -> b r j", b=16)
                )
                stv = stg[:].rearrange("b (r j) -> b j r", j=4)  # [16, 4, 8]

                # ---------- combine ----------
                s_tot = scrp.tile([16, 1], f32, tag="s_tot")
                nc.vector.reduce_sum(s_tot[:], stv[:, 0, :], axis=AX.X)
                q_tot = scrp.tile([16, 1], f32, tag="q_tot")
                nc.vector.reduce_sum(q_tot[:], stv[:, 1, :], axis=AX.X)
                m_tot = scrp.tile([16, 1], f32, tag="m_tot")
                nc.vector.reduce_max(m_tot[:], stv[:, 2, :], axis=AX.X)
                msk2 = scrp.tile([16, NCORE], mybir.dt.uint8, tag="msk2")
                nc.vector.tensor_scalar(msk2[:], stv[:, 2, :], m_tot[:], None,
                                        op0=ALU.is_equal)
                mi2 = scrp.tile([16, NCORE], f32, tag="mi2")
                nc.vector.select(mi2[:], msk2[:], stv[:, 3, :], bigv[:, 0:NCORE])
                pred = pr_all[:, t : t + 1]
                nc.vector.tensor_reduce(pred, mi2[:], op=ALU.min, axis=AX.X)
                idx_t = work.tile([16, 1], i32, tag="idx")
                nc.vector.tensor_copy(idx_t[:], pred)

                # ---------- LN1 scalars + unnormalized softmax ----------
                mu1 = scrp.tile([16, 1], f32, tag="mu1")
                nc.vector.tensor_scalar(mu1[:], s_tot[:], 1.0 / V, None, op0=ALU.mult)
                nmu2 = scrp.tile([16, 1], f32, tag="nmu2")
                nc.vector.scalar_tensor_tensor(nmu2[:], mu1[:], -1.0, mu1[:],
                                               op0=ALU.mult, op1=ALU.mult)
                var1 = scrp.tile([16, 1], f32, tag="var1")
                nc.vector.scalar_tensor_tensor(var1[:], q_tot[:], 1.0 / V, nmu2[:],
                                               op0=ALU.mult, op1=ALU.add)
                r1 = scrp.tile([16, 1], f32, tag="r1")
                nc.scalar.activation(r1[:], var1[:], AF.Sqrt, bias=epsv[:], scale=1.0)
                invs1 = scrp.tile([16, 1], f32, tag="invs1")
                nc.vector.reciprocal(invs1[:], r1[:])
                b1 = scrp.tile([16, 1], f32, tag="b1")
                nc.vector.scalar_tensor_tensor(b1[:], mu1[:], -1.0, invs1[:],
                                               op0=ALU.mult, op1=ALU.mult)
                ex = work.tile([16, VS], f32, tag="ex")
                nc.scalar.activation(ex[:], y1m[:, 0:VS], AF.Exp, bias=b1[:],
                                     scale=invs1[:], accum_out=se_all[:, t : t + 1])
                nc.sync.dma_start(q_out[t * 16 : (t + 1) * 16, :], ex[:])

                # ---------- next-step gathers ----------
                nc.gpsimd.indirect_dma_start(
                    ge_all[:, t * ES : (t + 1) * ES], None, embe_in[:],
                    IndirectOffsetOnAxis(ap=idx_t[:, 0:1], axis=0),
                )
                if t + 1 < t_steps:
                    nc.gpsimd.indirect_dma_start(
                        xg[:], None, embg_in[:],
                        IndirectOffsetOnAxis(ap=idx_t[:, 0:1], axis=0),
                    )

            # ---------- final output DMAs ----------
            nc.sync.dma_start(ge_out[:], ge_all[:])
            nc.sync.dma_start(se_out[:], se_all[:])
            nc.sync.dma_start(pr_out[:], pr_all[:])

    nc.compile()
    return nc


def _host_prep(inputs):
    fp = np.float32
    feats = np.asarray(inputs["features"], fp)
    emb_t = np.asarray(inputs["embed_table"], fp)
    W_ih = np.asarray(inputs["W_ih"], fp)
    W_hh = np.asarray(inputs["W_hh"], fp)
    b_ih = np.asarray(inputs["b_ih"], fp)
    b_hh = np.asarray(inputs["b_hh"], fp)
    lin_v = np.asarray(inputs["lin_v"], fp)
    lin_g = np.asarray(inputs["lin_g"], fp)
    lin1_v = np.asarray(inputs["lin1_v"], fp)
    lin1_g = np.asarray(inputs["lin1_g"], fp)

    W = (lin_g[:, None] * lin_v / np.linalg.norm(lin_v, axis=1, keepdims=True)).astype(fp)
    W1 = (lin1_g[:, None] * lin1_v / np.linalg.norm(lin1_v, axis=1, keepdims=True)).astype(fp)
    Wp = (W - W.mean(axis=0, keepdims=True)).astype(fp)
    G = Wp.astype(np.float64).T @ Wp.astype(np.float64)
    try:
        L = np.linalg.cholesky(G)
    except np.linalg.LinAlgError:
        L = np.linalg.cholesky(G + 1e-9 * np.eye(H))
    L = L.astype(fp)

    embgates = (emb_t @ W_ih.T + b_ih + b_hh).astype(fp)   # (V, 4H)
    gates0 = (feats @ W_ih.T + b_ih + b_hh).astype(fp)     # (B, 4H)

    sel = np.zeros((128, 16), fp)
    for j in range(4):
        for bb in range(16):
            sel[32 * j + bb, bb] = 1.0
    id16 = np.eye(16, dtype=fp)
    big = np.full((16, VS), 1e30, fp)

    in_maps = []
    for k in range(NCORE):
        rows = np.concatenate([
            np.arange(k * HS, (k + 1) * HS),                # i
            H + np.arange(k * HS, (k + 1) * HS),            # f
            3 * H + np.arange(k * HS, (k + 1) * HS),        # o
            2 * H + np.arange(k * HS, (k + 1) * HS),        # g
        ])
        whh_k = W_hh[rows]                                  # (GS, H)
        whhT = np.ascontiguousarray(
            whh_k.T.reshape(KH, 128, GS).transpose(1, 0, 2).reshape(128, KH * GS)
        )
        wp_k = Wp[k * VS : (k + 1) * VS]                    # (VS, H)
        wpT = np.ascontiguousarray(
            wp_k.T.reshape(KH, 128, VS).transpose(1, 0, 2).reshape(128, KH * VS)
        )
        lT = np.ascontiguousarray(
            L.T.reshape(KH, 128, H).transpose(1, 0, 2).reshape(128, KH * H)
        )
        w1_k = W1[k * VS : (k + 1) * VS]                    # (VS, V)
        w1T = np.ascontiguousarray(
            w1_k.T.reshape(KV, 128, VS).transpose(1, 0, 2).reshape(128, KV * VS)
        )
        iotag = np.broadcast_to(
            (k * VS + np.arange(VS)).astype(fp)[None, :], (16, VS)
        ).copy()
        in_maps.append({
            "whhT_in": whhT, "wpT_in": wpT, "lT_in": lT, "w1T_in": w1T,
            "embg_in": np.ascontiguousarray(embgates[:, rows]),
            "embe_in": np.ascontiguousarray(emb_t[:, k * ES : (k + 1) * ES]),
            "gates0_in": np.ascontiguousarray(gates0[:, rows]),
            "sel_in": sel, "id16_in": id16, "iotag_in": iotag, "big_in": big,
        })
    return in_maps


def kernel(**inputs):
    t_steps = int(np.asarray(inputs["captions"]).shape[1])
    in_maps = _host_prep(inputs)
    if t_steps not in _cache:
        _cache[t_steps] = _build(t_steps)
    nc = _cache[t_steps]
    import os as _os
    _tr = _os.environ.get("BASS_KERNEL_TRACE") == "1"
    res = run_bass_kernel_spmd(nc, in_maps, core_ids=list(range(NCORE)), trace=_tr)
    global LAST_RESULT
    LAST_RESULT = res
    rs = res.results

    fp = np.float32
    q_slices = np.stack([r["q_out"].reshape(t_steps, 16, VS) for r in rs], axis=0)
    q_full = q_slices.transpose(2, 1, 0, 3).reshape(16, t_steps, V)
    sumexp = np.sum(np.stack([r["se_out"] for r in rs]), axis=0)  # (16, T)
    Q = (q_full / sumexp[:, :, None]).astype(fp)
    hs = np.concatenate(
        [r["hs_out"].reshape(16, t_steps, HS) for r in rs], axis=2
    ).astype(fp)
    ge = np.concatenate(
        [r["ge_out"].reshape(16, t_steps, ES) for r in rs], axis=2
    ).astype(fp)
    preds = rs[0]["pr_out"].astype(fp)
    return (
        Q.reshape(B, t_steps, 1, 1, V),
        hs.reshape(B, t_steps, 1, 1, H),
        ge.reshape(B, t_steps, 1, 1, E),
        preds.reshape(B, t_steps, 1),
    )
